# revision 1
# baseline (speedup 1.0000x reference)
"""Segment-mean (word-pooling) kernel for Trainium2, 8 NeuronCores.

Problem: hidden_states [16, 4096, 768] f32, word_ids [16, 4096] i32
(non-decreasing per row, -1 = special token). Output [16, 2048, 768] f32:
mean of each word's subword embeddings; words with no tokens -> 0.

Strategy: pure data parallelism, 2 samples per core. Per sample, the
segment-mean is computed as a banded one-hot matmul on the PE:
  out[w, h] = sum_s onehot[s, w] * (1/count[w]) * x[s, h]
Tokens are processed in 32 k-tiles of 128; since word ids are
non-decreasing, each k-tile only touches a <=128-wide band of words, so
each k-tile contributes 1-2 matmuls into 128-word output windows
accumulated in PSUM. The one-hot (scaled by per-token reciprocal counts,
computed on host) is built on the vector engine with a single fused
is_equal*mult tensor_scalar op per k-tile against an iota ramp.

The SPMD program is identical on all 8 cores; the (k-tile, window)
pair structure is the union over samples, so per-core data that doesn't
touch a scheduled pair just contributes a zero one-hot block.
"""

import numpy as np

B, S, H = 16, 4096, 768
NUM_WORDS = S // 2  # 2048
N_CORES = 8
SPC = B // N_CORES  # samples per core = 2
P = 128
KT = S // P  # 32 k-tiles per sample
NW = NUM_WORDS // P  # 16 output windows per sample
NSPLITS = ((0, 512), (512, 768))  # matmul free-dim splits of H


def _plan(word_ids: np.ndarray):
    """Per-slot union plan. For each slot (0/1) and k-tile t: the window
    span [minwin, maxwin] over that slot's 8 samples; per window j the
    sorted member k-tiles. Returns (spans, members) per slot."""
    word_ids = np.minimum(word_ids, NUM_WORDS - 1)
    plans = []
    for slot in range(SPC):
        wid = word_ids[slot::SPC]  # the 8 samples this slot sees
        minwin = np.full(KT, NW, np.int64)
        maxwin = np.full(KT, -1, np.int64)
        for b in range(wid.shape[0]):
            row = wid[b]
            for t in range(KT):
                w = row[t * P : (t + 1) * P]
                w = w[w >= 0]
                if w.size:
                    minwin[t] = min(minwin[t], w.min() // P)
                    maxwin[t] = max(maxwin[t], w.max() // P)
        members = {j: [] for j in range(NW)}
        spans = []
        for t in range(KT):
            if maxwin[t] < 0:  # no valid token anywhere (can't happen)
                spans.append((0, 0))
                continue
            spans.append((int(minwin[t]), int(maxwin[t])))
            for j in range(int(minwin[t]), int(maxwin[t]) + 1):
                members[j].append(t)
        plans.append((spans, members))
    return plans


def _liveness(plans, in_group):
    """Max number of simultaneously-live x DMA groups / onehot tiles over
    the per-window emission order, across slots. A pool needs at least
    this many bufs or slot reuse can deadlock the DMA ring."""
    max_live_g, max_live_oh = 0, 0
    for spans, members in plans:
        first_g, last_g, first_oh, last_oh = {}, {}, {}, {}
        for j in range(NW):
            for t in members[j]:
                g = t // in_group
                first_g.setdefault(g, j)
                last_g[g] = j
                first_oh.setdefault(t, j)
                last_oh[t] = j
        for j in range(NW):
            live_g = sum(1 for g in first_g if first_g[g] <= j <= last_g[g])
            live_oh = sum(1 for t in first_oh if first_oh[t] <= j <= last_oh[t])
            max_live_g = max(max_live_g, live_g)
            max_live_oh = max(max_live_oh, live_oh)
    return max_live_g, max_live_oh


def _recip_counts(word_ids: np.ndarray) -> np.ndarray:
    """Per-token 1/count(word) as f32; 0 for special (-1) tokens."""
    r = np.zeros((B, S), np.float32)
    for b in range(B):
        wid = word_ids[b]
        valid = wid >= 0
        counts = np.bincount(wid[valid], minlength=NUM_WORDS)
        r[b, valid] = (1.0 / counts[wid[valid]]).astype(np.float32)
    return r


def _build(
    plans,
    reps=1,
    dyn_reps=1,
    do_mm=True,
    do_out=True,
    do_in=True,
    x_bufs=8,
    oh_bufs=8,
    ev_bufs=4,
    ps_bufs=3,
    in_group=4,
    out_group=2,
    in_alt=False,
    out_engine="scalar",
    in_dtype="f16",
    out_dtype="f16",
    n_splits=None,
    ev_engine="scalar",
    in_layout="rowmajor",
):
    """Build + compile the SPMD Bass program. reps>1 unrolls the whole
    body; dyn_reps>1 wraps it in a hardware For loop — both only used
    for amortized wall-clock timing. do_* flags ablate kernel stages
    for benchmarking (outputs are wrong when any is False)."""
    from contextlib import nullcontext
    import concourse.bacc as bacc
    import concourse.tile as tile
    from concourse import mybir

    nc = bacc.Bacc(
        "TRN2",
        target_bir_lowering=False,
        debug=False,
        enable_asserts=False,
        num_devices=N_CORES,
    )
    f32 = mybir.dt.float32
    fin = mybir.dt.float16 if in_dtype == "f16" else f32
    fout = mybir.dt.float16 if out_dtype == "f16" else f32
    if in_layout == "pmajor":
        x = nc.dram_tensor(
            "x", [SPC * KT // in_group, P, in_group * H], fin, kind="ExternalInput"
        ).ap()
    else:
        x = nc.dram_tensor("x", [SPC * S, H], fin, kind="ExternalInput").ap()
    widf = nc.dram_tensor("widf", [SPC, P, KT], f32, kind="ExternalInput").ap()
    rcp = nc.dram_tensor("rcp", [SPC, P, KT], f32, kind="ExternalInput").ap()
    y = nc.dram_tensor("y", [SPC * NUM_WORDS, H], fout, kind="ExternalOutput").ap()

    IOTA_W = NUM_WORDS + 2 * P  # ramp long enough for any window pair
    max_span = max(
        (jhi - jlo + 1) for spans, _ in plans for (jlo, jhi) in spans
    )

    # Size pools from plan liveness; degenerate plans (heavily overlapping
    # window k-ranges) fall back to a bounded reload mode, otherwise pool
    # slot reuse can deadlock the DMA ring.
    in_b = 2 if in_dtype == "f16" else 4
    live_g, live_oh = _liveness(plans, in_group)
    need_x, need_oh = live_g + 3, live_oh + 3
    x_bytes = need_x * in_group * H * in_b
    oh_bytes = need_oh * max_span * P * in_b
    safe = x_bytes + oh_bytes > 150 * 1024
    if not safe:
        x_bufs = max(x_bufs, need_x)
        oh_bufs = max(oh_bufs, need_oh)

    with tile.TileContext(nc) as tc:
        with (
            tc.tile_pool(name="const", bufs=1) as const_pool,
            tc.tile_pool(name="xin", bufs=x_bufs) as x_pool,
            tc.tile_pool(name="oh", bufs=oh_bufs) as oh_pool,
            tc.tile_pool(name="ev", bufs=ev_bufs) as ev_pool,
            tc.tile_pool(name="psum", bufs=ps_bufs, space="PSUM") as psum_pool,
        ):
            iota_i = const_pool.tile([P, IOTA_W], mybir.dt.int32)
            nc.gpsimd.iota(iota_i[:], pattern=[[1, IOTA_W]], base=0, channel_multiplier=0)
            iota_f = const_pool.tile([P, IOTA_W], f32)
            nc.vector.tensor_copy(out=iota_f[:], in_=iota_i[:])

            IG, OG = in_group, out_group
            out_eng = nc.sync if out_engine == "sync" else nc.scalar
            ev_eng = nc.vector if ev_engine == "vector" else nc.scalar
            splits = NSPLITS if n_splits is None else n_splits

            def emit(rep):
                for slot in range(SPC):
                    spans, members = plans[slot]
                    wid_t = const_pool.tile(
                        [P, KT], f32, name=f"wid_{rep}_{slot}", tag=f"wid{slot}"
                    )
                    nc.scalar.dma_start(out=wid_t[:], in_=widf[slot, :, :])
                    rcp_t = const_pool.tile(
                        [P, KT], f32, name=f"rcp_{rep}_{slot}", tag=f"rcp{slot}"
                    )
                    nc.scalar.dma_start(out=rcp_t[:], in_=rcp[slot, :, :])

                    xg_tiles = {}
                    oh_tiles = {}

                    def get_x(t):
                        g, a = divmod(t, IG)
                        if g not in xg_tiles:
                            xt = x_pool.tile(
                                [P, IG, H], fin, name=f"xt_{rep}_{slot}_{g}", tag="xt"
                            )
                            if do_in:
                                if in_layout == "pmajor":
                                    src = x[slot * (KT // IG) + g, :, :].rearrange(
                                        "p (a h) -> p a h", a=IG
                                    )
                                else:
                                    r0 = slot * S + g * IG * P
                                    src = x[r0 : r0 + IG * P, :].rearrange(
                                        "(a p) h -> p a h", p=P
                                    )
                                eng = (
                                    nc.scalar
                                    if (in_alt and g % 2 == 1)
                                    else nc.sync
                                )
                                eng.dma_start(out=xt[:], in_=src)
                            xg_tiles[g] = xt
                        return xg_tiles[g][:, t % IG, :]

                    def get_oh(t):
                        if t not in oh_tiles:
                            jlo, jhi = spans[t]
                            wspan = (jhi - jlo + 1) * P
                            oh = oh_pool.tile(
                                [P, max_span * P],
                                fin,
                                name=f"oh_{rep}_{slot}_{t}",
                                tag="oh",
                            )
                            nc.vector.tensor_scalar(
                                out=oh[:, :wspan],
                                in0=iota_f[:, jlo * P : jlo * P + wspan],
                                scalar1=wid_t[:, t : t + 1],
                                scalar2=rcp_t[:, t : t + 1],
                                op0=mybir.AluOpType.is_equal,
                                op1=mybir.AluOpType.mult,
                            )
                            oh_tiles[t] = oh
                        return oh_tiles[t]

                    og_tile = [None]

                    for j in range(NW):
                        if j % OG == 0:
                            og_tile[0] = ev_pool.tile(
                                [P, OG, H], fout, name=f"out_{rep}_{slot}_{j}", tag="out"
                            )
                        out_sb = og_tile[0][:, j % OG, :]
                        ks = members[j]
                        if not do_mm:
                            for t in ks:
                                get_x(t)
                                get_oh(t)
                        if not ks:
                            nc.vector.memset(out_sb, 0.0)
                        elif not do_mm:
                            if do_out:
                                nc.gpsimd.memset(out_sb, 0.0)
                        else:
                            ps = psum_pool.tile(
                                [P, H], f32, name=f"ps_{rep}_{slot}_{j}", tag="ps"
                            )
                            for ki, t in enumerate(ks):
                                if safe:
                                    xs = x_pool.tile(
                                        [P, 1, H], fin,
                                        name=f"xs_{rep}_{slot}_{j}_{t}", tag="xt",
                                    )
                                    r0 = slot * S + t * P
                                    nc.sync.dma_start(
                                        out=xs[:], in_=x[r0 : r0 + P, :].rearrange(
                                            "(a p) h -> p a h", p=P
                                        )
                                    )
                                    xt = xs[:, 0, :]
                                    oh = oh_pool.tile(
                                        [P, P], fin,
                                        name=f"ohs_{rep}_{slot}_{j}_{t}", tag="oh",
                                    )
                                    nc.vector.tensor_scalar(
                                        out=oh[:, :],
                                        in0=iota_f[:, j * P : (j + 1) * P],
                                        scalar1=wid_t[:, t : t + 1],
                                        scalar2=rcp_t[:, t : t + 1],
                                        op0=mybir.AluOpType.is_equal,
                                        op1=mybir.AluOpType.mult,
                                    )
                                    off = 0
                                else:
                                    xt = get_x(t)
                                    oh = get_oh(t)
                                    off = (j - spans[t][0]) * P
                                for lo, hi in splits:
                                    nc.tensor.matmul(
                                        out=ps[:, lo:hi],
                                        lhsT=oh[:, off : off + P],
                                        rhs=xt[:, lo:hi],
                                        start=(ki == 0),
                                        stop=(ki == len(ks) - 1),
                                    )
                            if ev_engine == "vector":
                                nc.vector.tensor_copy(out=out_sb, in_=ps[:])
                            else:
                                nc.scalar.copy(out=out_sb, in_=ps[:])
                        if do_out and j % OG == OG - 1:
                            r0 = slot * NUM_WORDS + (j - OG + 1) * P
                            dst = y[r0 : r0 + OG * P, :].rearrange(
                                "(a p) h -> p a h", p=P
                            )
                            oe = (
                                (nc.scalar if (j // OG) % 2 == 0 else nc.sync)
                                if out_engine == "alt"
                                else out_eng
                            )
                            oe.dma_start(out=dst, in_=og_tile[0][:])

            loop_cm = (
                tc.For_i(0, dyn_reps, 1) if dyn_reps > 1 else nullcontext()
            )
            with loop_cm:
                for rep in range(reps):
                    emit(rep)

    nc.compile()
    return nc


def _prep_inputs(hidden_states, word_ids, in_dtype="f16", in_layout="rowmajor",
                 in_group=4):
    np_in = np.float16 if in_dtype == "f16" else np.float32
    hs = np.ascontiguousarray(np.asarray(hidden_states, dtype=np_in))
    wid = np.minimum(np.asarray(word_ids, dtype=np.int32), NUM_WORDS - 1)
    assert hs.shape == (B, S, H) and wid.shape == (B, S)
    r = _recip_counts(wid)
    # [B, S] -> [B, P, KT]: element (p, t) = token t*P + p
    widf = np.ascontiguousarray(
        wid.astype(np.float32).reshape(B, KT, P).transpose(0, 2, 1)
    )
    rt = np.ascontiguousarray(r.reshape(B, KT, P).transpose(0, 2, 1))
    in_maps = []
    for c in range(N_CORES):
        sl = slice(c * SPC, (c + 1) * SPC)
        if in_layout == "pmajor":
            IG = in_group
            xc = np.ascontiguousarray(
                hs[sl]
                .reshape(SPC, KT // IG, IG, P, H)
                .transpose(0, 1, 3, 2, 4)
                .reshape(SPC * KT // IG, P, IG * H)
            )
        else:
            xc = hs[sl].reshape(SPC * S, H)
        in_maps.append({"x": xc, "widf": widf[sl], "rcp": rt[sl]})
    return in_maps


def kernel(hidden_states, word_ids):
    import concourse.bass_utils as bass_utils

    wid = np.asarray(word_ids, dtype=np.int32)
    plans = _plan(wid)
    nc = _build(plans)
    in_maps = _prep_inputs(hidden_states, word_ids)
    res = bass_utils.run_bass_kernel_spmd(nc, in_maps, core_ids=list(range(N_CORES)))
    out = np.empty((B, NUM_WORDS, H), np.float32)
    for c in range(N_CORES):
        yc = np.asarray(res.results[c]["y"], dtype=np.float32)
        for slot in range(SPC):
            out[c * SPC + slot] = yc[slot * NUM_WORDS : (slot + 1) * NUM_WORDS]
    return out



# revision 31
# speedup vs baseline: 2.5153x; 2.5153x over previous
"""Segment-mean (word-pooling) kernel for Trainium2, 8 NeuronCores.

Problem: hidden_states [16, 4096, 768] f32, word_ids [16, 4096] i32
(non-decreasing per row, -1 = special token). Output [16, 2048, 768] f32:
mean of each word's subword embeddings; words with no tokens -> 0.

Strategy: pure data parallelism, 2 samples per core. Per sample, the
segment-mean is computed as a banded one-hot matmul on the PE:
  out[w, h] = sum_s onehot[s, w] * (1/count[w]) * x[s, h]
Tokens are processed in 32 k-tiles of 128; since word ids are
non-decreasing, each k-tile only touches a <=128-wide band of words, so
each k-tile contributes 1-2 matmuls into 128-word output windows
accumulated in PSUM. In the default "tt" mode the one-hot is a pure 0/1
mask built on the vector engine with one tensor_tensor is_equal per
k-tile (wid column broadcast along the free dim vs an iota ramp; exact
in f16), and the per-word 1/count is folded into the PSUM->SBUF
eviction as a per-partition activation scale on the scalar engine.
The legacy "ptr" mode fused is_equal*mult with per-partition scalar
pointers on the vector engine instead.

Measured on hw (R256/R2048 amortized): the kernel is DMA-roofline
bound; in+out f16 traffic is 18.9 MB/core and the full kernel time
equals the pure-DMA ablation time, with PE (~52us chain) and
DVE/ACT work fully hidden. Layout variants (pmajor contiguous input,
grouped/pmajor output, queue alternation) all measured neutral-to-worse
than this rowmajor IG=4/OG=2 configuration.

The SPMD program is identical on all 8 cores; the (k-tile, window)
pair structure is the union over samples, so per-core data that doesn't
touch a scheduled pair just contributes a zero one-hot block.
"""

import numpy as np

B, S, H = 16, 4096, 768
NUM_WORDS = S // 2  # 2048
N_CORES = 8
SPC = B // N_CORES  # samples per core = 2
P = 128
KT = S // P  # 32 k-tiles per sample
NW = NUM_WORDS // P  # 16 output windows per sample
NSPLITS = ((0, 512), (512, 768))  # matmul free-dim splits of H


def _plan(word_ids: np.ndarray):
    """Per-slot union plan. For each slot (0/1) and k-tile t: the window
    span [minwin, maxwin] over that slot's 8 samples; per window j the
    sorted member k-tiles. Returns (spans, members) per slot."""
    word_ids = np.minimum(word_ids, NUM_WORDS - 1)
    plans = []
    for slot in range(SPC):
        wid = word_ids[slot::SPC]  # the 8 samples this slot sees
        minwin = np.full(KT, NW, np.int64)
        maxwin = np.full(KT, -1, np.int64)
        for b in range(wid.shape[0]):
            row = wid[b]
            for t in range(KT):
                w = row[t * P : (t + 1) * P]
                w = w[w >= 0]
                if w.size:
                    minwin[t] = min(minwin[t], w.min() // P)
                    maxwin[t] = max(maxwin[t], w.max() // P)
        members = {j: [] for j in range(NW)}
        spans = []
        for t in range(KT):
            if maxwin[t] < 0:  # no valid token anywhere (can't happen)
                spans.append((0, 0))
                continue
            spans.append((int(minwin[t]), int(maxwin[t])))
            for j in range(int(minwin[t]), int(maxwin[t]) + 1):
                members[j].append(t)
        plans.append((spans, members))
    return plans


def _liveness(plans, in_group):
    """Max number of simultaneously-live x DMA groups / onehot tiles over
    the per-window emission order, across slots. A pool needs at least
    this many bufs or slot reuse can deadlock the DMA ring."""
    max_live_g, max_live_oh = 0, 0
    for spans, members in plans:
        first_g, last_g, first_oh, last_oh = {}, {}, {}, {}
        for j in range(NW):
            for t in members[j]:
                g = t // in_group
                first_g.setdefault(g, j)
                last_g[g] = j
                first_oh.setdefault(t, j)
                last_oh[t] = j
        for j in range(NW):
            live_g = sum(1 for g in first_g if first_g[g] <= j <= last_g[g])
            live_oh = sum(1 for t in first_oh if first_oh[t] <= j <= last_oh[t])
            max_live_g = max(max_live_g, live_g)
            max_live_oh = max(max_live_oh, live_oh)
    return max_live_g, max_live_oh


def _recip_counts(word_ids: np.ndarray) -> np.ndarray:
    """Per-token 1/count(word) as f32; 0 for special (-1) tokens."""
    r = np.zeros((B, S), np.float32)
    for b in range(B):
        wid = word_ids[b]
        valid = wid >= 0
        counts = np.bincount(wid[valid], minlength=NUM_WORDS)
        r[b, valid] = (1.0 / counts[wid[valid]]).astype(np.float32)
    return r


def _build(
    plans,
    reps=1,
    dyn_reps=1,
    do_mm=True,
    do_out=True,
    do_in=True,
    do_ev=True,
    x_bufs=8,
    oh_bufs=8,
    ev_bufs=4,
    ps_bufs=3,
    in_group=4,
    out_group=2,
    in_alt=False,
    out_engine="scalar",
    in_dtype="f16",
    out_dtype="f16",
    n_splits=None,
    ev_engine="scalar",
    in_layout="rowmajor",
    out_layout="rowmajor",
    wid_engine="scalar",
    oh_dtype="f32",
    oh_mode="ptr",
    oh_probe=None,
):
    """Build + compile the SPMD Bass program. reps>1 unrolls the whole
    body; dyn_reps>1 wraps it in a hardware For loop — both only used
    for amortized wall-clock timing. do_* flags ablate kernel stages
    for benchmarking (outputs are wrong when any is False)."""
    from contextlib import nullcontext
    import concourse.bacc as bacc
    import concourse.tile as tile
    from concourse import mybir

    nc = bacc.Bacc(
        "TRN2",
        target_bir_lowering=False,
        debug=False,
        enable_asserts=False,
        num_devices=N_CORES,
    )
    f32 = mybir.dt.float32
    _dtmap = {"f16": mybir.dt.float16, "bf16": mybir.dt.bfloat16, "f32": f32}
    fin = _dtmap[in_dtype]
    fout = _dtmap[out_dtype]
    if in_layout == "pmajor":
        x = nc.dram_tensor(
            "x", [SPC * KT // in_group, P, in_group * H], fin, kind="ExternalInput"
        ).ap()
    else:
        x = nc.dram_tensor("x", [SPC * S, H], fin, kind="ExternalInput").ap()
    foh = _dtmap[oh_dtype]
    widf = nc.dram_tensor("widf", [SPC, P, KT], foh, kind="ExternalInput").ap()
    if oh_mode == "tt":
        # per-word reciprocal counts, applied as per-partition scale on evict
        rcpw = nc.dram_tensor("rcpw", [SPC, P, NW], f32, kind="ExternalInput").ap()
    else:
        rcp = nc.dram_tensor("rcp", [SPC, P, KT], foh, kind="ExternalInput").ap()
    if out_layout == "pmajor":
        y = nc.dram_tensor(
            "y", [SPC * NW // out_group, P, out_group * H], fout, kind="ExternalOutput"
        ).ap()
    else:
        y = nc.dram_tensor("y", [SPC * NUM_WORDS, H], fout, kind="ExternalOutput").ap()

    IOTA_W = NUM_WORDS + 2 * P  # ramp long enough for any window pair
    max_span = max(
        (jhi - jlo + 1) for spans, _ in plans for (jlo, jhi) in spans
    )

    # Size pools from plan liveness; degenerate plans (heavily overlapping
    # window k-ranges) fall back to a bounded reload mode, otherwise pool
    # slot reuse can deadlock the DMA ring.
    in_b = 2 if in_dtype == "f16" else 4
    live_g, live_oh = _liveness(plans, in_group)
    need_x, need_oh = live_g + 3, live_oh + 3
    x_bytes = need_x * in_group * H * in_b
    oh_bytes = need_oh * max_span * P * in_b
    safe = x_bytes + oh_bytes > 150 * 1024
    if not safe:
        x_bufs = max(x_bufs, need_x)
        oh_bufs = max(oh_bufs, need_oh)

    with tile.TileContext(nc) as tc:
        with (
            tc.tile_pool(name="const", bufs=1) as const_pool,
            tc.tile_pool(name="xin", bufs=x_bufs) as x_pool,
            tc.tile_pool(name="oh", bufs=oh_bufs) as oh_pool,
            tc.tile_pool(name="ev", bufs=ev_bufs) as ev_pool,
            tc.tile_pool(name="psum", bufs=ps_bufs, space="PSUM") as psum_pool,
        ):
            iota_i = const_pool.tile([P, IOTA_W], mybir.dt.int32)
            nc.gpsimd.iota(iota_i[:], pattern=[[1, IOTA_W]], base=0, channel_multiplier=0)
            iota_f = const_pool.tile([P, IOTA_W], foh)
            nc.vector.tensor_copy(out=iota_f[:], in_=iota_i[:])

            dummy_x = None
            if not do_in:
                dummy_x = const_pool.tile([P, in_group, H], fin)
                nc.vector.memset(dummy_x[:], 0.0)

            IG, OG = in_group, out_group
            out_eng = nc.sync if out_engine == "sync" else nc.scalar
            ev_eng = nc.vector if ev_engine == "vector" else nc.scalar
            splits = NSPLITS if n_splits is None else n_splits

            def emit(rep):
                for slot in range(SPC):
                    spans, members = plans[slot]
                    wid_eng = nc.sync if wid_engine == "sync" else nc.scalar
                    wid_t = const_pool.tile(
                        [P, KT], foh, name=f"wid_{rep}_{slot}", tag=f"wid{slot}"
                    )
                    wid_eng.dma_start(out=wid_t[:], in_=widf[slot, :, :])
                    if oh_mode == "tt":
                        rcpw_t = const_pool.tile(
                            [P, NW], f32, name=f"rcpw_{rep}_{slot}", tag=f"rcpw{slot}"
                        )
                        wid_eng.dma_start(out=rcpw_t[:], in_=rcpw[slot, :, :])
                    else:
                        rcp_t = const_pool.tile(
                            [P, KT], foh, name=f"rcp_{rep}_{slot}", tag=f"rcp{slot}"
                        )
                        wid_eng.dma_start(out=rcp_t[:], in_=rcp[slot, :, :])

                    xg_tiles = {}
                    oh_tiles = {}

                    def get_x(t):
                        if not do_in:
                            return dummy_x[:, t % IG, :]
                        g, a = divmod(t, IG)
                        if g not in xg_tiles:
                            xt = x_pool.tile(
                                [P, IG, H], fin, name=f"xt_{rep}_{slot}_{g}", tag="xt"
                            )
                            if do_in:
                                if in_layout == "pmajor":
                                    src = x[slot * (KT // IG) + g, :, :].rearrange(
                                        "p (a h) -> p a h", a=IG
                                    )
                                else:
                                    r0 = slot * S + g * IG * P
                                    src = x[r0 : r0 + IG * P, :].rearrange(
                                        "(a p) h -> p a h", p=P
                                    )
                                eng = (
                                    nc.scalar
                                    if (in_alt and g % 2 == 1)
                                    else nc.sync
                                )
                                eng.dma_start(out=xt[:], in_=src)
                            xg_tiles[g] = xt
                        return xg_tiles[g][:, t % IG, :]

                    def get_oh(t):
                        if t not in oh_tiles:
                            jlo, jhi = spans[t]
                            wspan = (jhi - jlo + 1) * P
                            oh = oh_pool.tile(
                                [P, max_span * P],
                                fin,
                                name=f"oh_{rep}_{slot}_{t}",
                                tag="oh",
                            )
                            if oh_mode == "tt":
                                nc.vector.tensor_tensor(
                                    out=oh[:, :wspan],
                                    in0=wid_t[:, t : t + 1].to_broadcast((P, wspan)),
                                    in1=iota_f[:, jlo * P : jlo * P + wspan],
                                    op=mybir.AluOpType.is_equal,
                                )
                            else:
                                nc.vector.tensor_scalar(
                                    out=oh[:, :wspan],
                                    in0=iota_f[:, jlo * P : jlo * P + wspan],
                                    scalar1=wid_t[:, t : t + 1],
                                    scalar2=rcp_t[:, t : t + 1],
                                    op0=mybir.AluOpType.is_equal,
                                    op1=mybir.AluOpType.mult,
                                )
                            oh_tiles[t] = oh
                        return oh_tiles[t]

                    og_tile = [None]

                    for j in range(NW):
                        if do_ev and j % OG == 0:
                            og_tile[0] = ev_pool.tile(
                                [P, OG, H], fout, name=f"out_{rep}_{slot}_{j}", tag="out"
                            )
                        out_sb = og_tile[0][:, j % OG, :] if do_ev else None
                        ks = members[j]
                        if not do_mm:
                            for t in ks:
                                get_x(t)
                                if oh_probe != "skip":
                                    get_oh(t)
                        if not ks:
                            if do_ev:
                                nc.vector.memset(out_sb, 0.0)
                        elif not do_mm:
                            if do_ev and do_out:
                                nc.vector.memset(out_sb, 0.0)
                        else:
                            ps = psum_pool.tile(
                                [P, H], f32, name=f"ps_{rep}_{slot}_{j}", tag="ps"
                            )
                            for ki, t in enumerate(ks):
                                if safe:
                                    xs = x_pool.tile(
                                        [P, 1, H], fin,
                                        name=f"xs_{rep}_{slot}_{j}_{t}", tag="xt",
                                    )
                                    r0 = slot * S + t * P
                                    nc.sync.dma_start(
                                        out=xs[:], in_=x[r0 : r0 + P, :].rearrange(
                                            "(a p) h -> p a h", p=P
                                        )
                                    )
                                    xt = xs[:, 0, :]
                                    oh = oh_pool.tile(
                                        [P, P], fin,
                                        name=f"ohs_{rep}_{slot}_{j}_{t}", tag="oh",
                                    )
                                    if oh_mode == "tt":
                                        nc.vector.tensor_tensor(
                                            out=oh[:, :],
                                            in0=wid_t[:, t : t + 1].to_broadcast(
                                                (P, P)
                                            ),
                                            in1=iota_f[:, j * P : (j + 1) * P],
                                            op=mybir.AluOpType.is_equal,
                                        )
                                    else:
                                        nc.vector.tensor_scalar(
                                            out=oh[:, :],
                                            in0=iota_f[:, j * P : (j + 1) * P],
                                            scalar1=wid_t[:, t : t + 1],
                                            scalar2=rcp_t[:, t : t + 1],
                                            op0=mybir.AluOpType.is_equal,
                                            op1=mybir.AluOpType.mult,
                                        )
                                    off = 0
                                else:
                                    xt = get_x(t)
                                    oh = get_oh(t)
                                    off = (j - spans[t][0]) * P
                                for lo, hi in splits:
                                    nc.tensor.matmul(
                                        out=ps[:, lo:hi],
                                        lhsT=oh[:, off : off + P],
                                        rhs=xt[:, lo:hi],
                                        start=(ki == 0),
                                        stop=(ki == len(ks) - 1),
                                    )
                            if not do_ev:
                                pass
                            elif ev_engine == "vector" or (
                                ev_engine == "alt" and j % 2 == 1
                            ):
                                if oh_mode == "tt":
                                    nc.vector.tensor_tensor(
                                        out=out_sb,
                                        in0=ps[:],
                                        in1=rcpw_t[:, j : j + 1].to_broadcast((P, H)),
                                        op=mybir.AluOpType.mult,
                                    )
                                else:
                                    nc.vector.tensor_copy(out=out_sb, in_=ps[:])
                            else:
                                if oh_mode == "tt":
                                    nc.scalar.activation(
                                        out=out_sb,
                                        in_=ps[:],
                                        func=mybir.ActivationFunctionType.Copy,
                                        scale=rcpw_t[:, j : j + 1],
                                    )
                                else:
                                    nc.scalar.copy(out=out_sb, in_=ps[:])
                        if do_ev and do_out and j % OG == OG - 1:
                            if out_layout == "pmajor":
                                dst = y[slot * (NW // OG) + j // OG, :, :].rearrange(
                                    "p (a h) -> p a h", a=OG
                                )
                            else:
                                r0 = slot * NUM_WORDS + (j - OG + 1) * P
                                dst = y[r0 : r0 + OG * P, :].rearrange(
                                    "(a p) h -> p a h", p=P
                                )
                            oe = (
                                (nc.scalar if (j // OG) % 2 == 0 else nc.sync)
                                if out_engine == "alt"
                                else out_eng
                            )
                            oe.dma_start(out=dst, in_=og_tile[0][:])

            loop_cm = (
                tc.For_i(0, dyn_reps, 1) if dyn_reps > 1 else nullcontext()
            )
            with loop_cm:
                for rep in range(reps):
                    emit(rep)

    nc.compile()
    return nc


def _word_recips(word_ids: np.ndarray) -> np.ndarray:
    """[B, P, NW] f32: 1/count for word j*P+p of sample b (1 if empty)."""
    out = np.ones((B, NUM_WORDS), np.float32)
    for b in range(B):
        wid = word_ids[b]
        counts = np.bincount(wid[wid >= 0], minlength=NUM_WORDS)
        nz = counts > 0
        out[b, nz] = 1.0 / counts[nz]
    return np.ascontiguousarray(
        out.reshape(B, NW, P).transpose(0, 2, 1).astype(np.float32)
    )


def _prep_inputs(hidden_states, word_ids, in_dtype="f16", in_layout="rowmajor",
                 in_group=4, oh_dtype="f32", oh_mode="ptr"):
    if in_dtype == "bf16":
        import ml_dtypes

        np_in = ml_dtypes.bfloat16
    else:
        np_in = np.float16 if in_dtype == "f16" else np.float32
    np_oh = np.float16 if oh_dtype == "f16" else np.float32
    hs = np.ascontiguousarray(np.asarray(hidden_states, dtype=np_in))
    wid = np.minimum(np.asarray(word_ids, dtype=np.int32), NUM_WORDS - 1)
    assert hs.shape == (B, S, H) and wid.shape == (B, S)
    r = _recip_counts(wid)
    # [B, S] -> [B, P, KT]: element (p, t) = token t*P + p
    widf = np.ascontiguousarray(
        wid.astype(np_oh).reshape(B, KT, P).transpose(0, 2, 1)
    )
    rt = np.ascontiguousarray(r.astype(np_oh).reshape(B, KT, P).transpose(0, 2, 1))
    rcpw = _word_recips(wid) if oh_mode == "tt" else None
    in_maps = []
    for c in range(N_CORES):
        sl = slice(c * SPC, (c + 1) * SPC)
        if in_layout == "pmajor":
            IG = in_group
            xc = np.ascontiguousarray(
                hs[sl]
                .reshape(SPC, KT // IG, IG, P, H)
                .transpose(0, 1, 3, 2, 4)
                .reshape(SPC * KT // IG, P, IG * H)
            )
        else:
            xc = hs[sl].reshape(SPC * S, H)
        m = {"x": xc, "widf": widf[sl]}
        if oh_mode == "tt":
            m["rcpw"] = rcpw[sl]
        else:
            m["rcp"] = rt[sl]
        in_maps.append(m)
    return in_maps


# config used by kernel() — the graded entry point
BUILD_KW = {"oh_mode": "tt", "oh_dtype": "f16"}
PREP_KW = {"oh_mode": "tt", "oh_dtype": "f16"}


def kernel(hidden_states, word_ids):
    import concourse.bass_utils as bass_utils

    wid = np.asarray(word_ids, dtype=np.int32)
    plans = _plan(wid)
    nc = _build(plans, **BUILD_KW)
    in_maps = _prep_inputs(hidden_states, word_ids, **PREP_KW)
    res = bass_utils.run_bass_kernel_spmd(nc, in_maps, core_ids=list(range(N_CORES)))
    out = np.empty((B, NUM_WORDS, H), np.float32)
    OG = BUILD_KW.get("out_group", 2)
    pmajor_out = BUILD_KW.get("out_layout", "rowmajor") == "pmajor"
    for c in range(N_CORES):
        yc = np.asarray(res.results[c]["y"], dtype=np.float32)
        if pmajor_out:
            yc = (
                yc.reshape(SPC, NW // OG, P, OG, H)
                .transpose(0, 1, 3, 2, 4)
                .reshape(SPC, NUM_WORDS, H)
            )
            for slot in range(SPC):
                out[c * SPC + slot] = yc[slot]
        else:
            for slot in range(SPC):
                out[c * SPC + slot] = yc[slot * NUM_WORDS : (slot + 1) * NUM_WORDS]
    return out



# revision 34
# speedup vs baseline: 2.7462x; 1.0918x over previous
"""Segment-mean (word-pooling) kernel for Trainium2, 8 NeuronCores.

Problem: hidden_states [16, 4096, 768] f32, word_ids [16, 4096] i32
(non-decreasing per row, -1 = special token). Output [16, 2048, 768] f32:
mean of each word's subword embeddings; words with no tokens -> 0.

Strategy: pure data parallelism, 2 samples per core. Per sample, the
segment-mean is computed as a banded one-hot matmul on the PE:
  out[w, h] = sum_s onehot[s, w] * (1/count[w]) * x[s, h]
Tokens are processed in 32 k-tiles of 128; since word ids are
non-decreasing, each k-tile only touches a <=128-wide band of words, so
each k-tile contributes 1-2 matmuls into 128-word output windows
accumulated in PSUM. In the default "tt" mode the one-hot is a pure 0/1
mask built on the vector engine with one tensor_tensor is_equal per
k-tile (wid column broadcast along the free dim vs an iota ramp; exact
in f16), and the per-word 1/count is folded into the PSUM->SBUF
eviction as a per-partition activation scale on the scalar engine.
The legacy "ptr" mode fused is_equal*mult with per-partition scalar
pointers on the vector engine instead.

Measured on hw (R256/R2048 amortized): the kernel is DMA-roofline
bound; in+out f16 traffic is 18.9 MB/core and the full kernel time
equals the pure-DMA ablation time, with PE (~52us chain) and
DVE/ACT work fully hidden. Layout variants (pmajor contiguous input,
grouped/pmajor output, queue alternation) all measured neutral-to-worse
than this rowmajor IG=4/OG=2 configuration.

The SPMD program is identical on all 8 cores; the (k-tile, window)
pair structure is the union over samples, so per-core data that doesn't
touch a scheduled pair just contributes a zero one-hot block.
"""

import numpy as np

B, S, H = 16, 4096, 768
NUM_WORDS = S // 2  # 2048
N_CORES = 8
SPC = B // N_CORES  # samples per core = 2
P = 128
KT = S // P  # 32 k-tiles per sample
NW = NUM_WORDS // P  # 16 output windows per sample
NSPLITS = ((0, 512), (512, 768))  # matmul free-dim splits of H


def _plan(word_ids: np.ndarray):
    """Per-slot union plan. For each slot (0/1) and k-tile t: the window
    span [minwin, maxwin] over that slot's 8 samples; per window j the
    sorted member k-tiles. Returns (spans, members) per slot."""
    word_ids = np.minimum(word_ids, NUM_WORDS - 1)
    plans = []
    for slot in range(SPC):
        wid = word_ids[slot::SPC]  # the 8 samples this slot sees
        minwin = np.full(KT, NW, np.int64)
        maxwin = np.full(KT, -1, np.int64)
        for b in range(wid.shape[0]):
            row = wid[b]
            for t in range(KT):
                w = row[t * P : (t + 1) * P]
                w = w[w >= 0]
                if w.size:
                    minwin[t] = min(minwin[t], w.min() // P)
                    maxwin[t] = max(maxwin[t], w.max() // P)
        members = {j: [] for j in range(NW)}
        spans = []
        for t in range(KT):
            if maxwin[t] < 0:  # no valid token anywhere (can't happen)
                spans.append((0, 0))
                continue
            spans.append((int(minwin[t]), int(maxwin[t])))
            for j in range(int(minwin[t]), int(maxwin[t]) + 1):
                members[j].append(t)
        plans.append((spans, members))
    return plans


def _liveness(plans, in_group):
    """Max number of simultaneously-live x DMA groups / onehot tiles over
    the per-window emission order, across slots. A pool needs at least
    this many bufs or slot reuse can deadlock the DMA ring."""
    max_live_g, max_live_oh = 0, 0
    for spans, members in plans:
        first_g, last_g, first_oh, last_oh = {}, {}, {}, {}
        for j in range(NW):
            for t in members[j]:
                g = t // in_group
                first_g.setdefault(g, j)
                last_g[g] = j
                first_oh.setdefault(t, j)
                last_oh[t] = j
        for j in range(NW):
            live_g = sum(1 for g in first_g if first_g[g] <= j <= last_g[g])
            live_oh = sum(1 for t in first_oh if first_oh[t] <= j <= last_oh[t])
            max_live_g = max(max_live_g, live_g)
            max_live_oh = max(max_live_oh, live_oh)
    return max_live_g, max_live_oh


def _recip_counts(word_ids: np.ndarray) -> np.ndarray:
    """Per-token 1/count(word) as f32; 0 for special (-1) tokens."""
    r = np.zeros((B, S), np.float32)
    for b in range(B):
        wid = word_ids[b]
        valid = wid >= 0
        counts = np.bincount(wid[valid], minlength=NUM_WORDS)
        r[b, valid] = (1.0 / counts[wid[valid]]).astype(np.float32)
    return r


def _build(
    plans,
    reps=1,
    dyn_reps=1,
    do_mm=True,
    do_out=True,
    do_in=True,
    do_ev=True,
    x_bufs=8,
    oh_bufs=8,
    ev_bufs=4,
    ps_bufs=3,
    in_group=4,
    out_group=2,
    in_alt=False,
    out_engine="scalar",
    in_dtype="f16",
    out_dtype="f16",
    n_splits=None,
    ev_engine="scalar",
    in_layout="rowmajor",
    out_layout="rowmajor",
    wid_engine="scalar",
    oh_dtype="f32",
    oh_mode="ptr",
    oh_probe=None,
    stagger=False,
):
    """Build + compile the SPMD Bass program. reps>1 unrolls the whole
    body; dyn_reps>1 wraps it in a hardware For loop — both only used
    for amortized wall-clock timing. do_* flags ablate kernel stages
    for benchmarking (outputs are wrong when any is False)."""
    from contextlib import nullcontext
    import concourse.bacc as bacc
    import concourse.tile as tile
    from concourse import mybir

    nc = bacc.Bacc(
        "TRN2",
        target_bir_lowering=False,
        debug=False,
        enable_asserts=False,
        num_devices=N_CORES,
    )
    f32 = mybir.dt.float32
    _dtmap = {"f16": mybir.dt.float16, "bf16": mybir.dt.bfloat16, "f32": f32}
    fin = _dtmap[in_dtype]
    fout = _dtmap[out_dtype]
    if in_layout == "pmajor":
        x = nc.dram_tensor(
            "x", [SPC * KT // in_group, P, in_group * H], fin, kind="ExternalInput"
        ).ap()
    else:
        x = nc.dram_tensor("x", [SPC * S, H], fin, kind="ExternalInput").ap()
    foh = _dtmap[oh_dtype]
    widf = nc.dram_tensor("widf", [SPC, P, KT], foh, kind="ExternalInput").ap()
    if oh_mode == "tt":
        # per-word reciprocal counts, applied as per-partition scale on evict
        rcpw = nc.dram_tensor("rcpw", [SPC, P, NW], f32, kind="ExternalInput").ap()
    else:
        rcp = nc.dram_tensor("rcp", [SPC, P, KT], foh, kind="ExternalInput").ap()
    if out_layout == "pmajor":
        y = nc.dram_tensor(
            "y", [SPC * NW // out_group, P, out_group * H], fout, kind="ExternalOutput"
        ).ap()
    else:
        y = nc.dram_tensor("y", [SPC * NUM_WORDS, H], fout, kind="ExternalOutput").ap()

    IOTA_W = NUM_WORDS + 2 * P  # ramp long enough for any window pair
    max_span = max(
        (jhi - jlo + 1) for spans, _ in plans for (jlo, jhi) in spans
    )

    # Size pools from plan liveness; degenerate plans (heavily overlapping
    # window k-ranges) fall back to a bounded reload mode, otherwise pool
    # slot reuse can deadlock the DMA ring.
    in_b = 2 if in_dtype == "f16" else 4
    live_g, live_oh = _liveness(plans, in_group)
    need_x, need_oh = live_g + 3, live_oh + 3
    x_bytes = need_x * in_group * H * in_b
    oh_bytes = need_oh * max_span * P * in_b
    safe = x_bytes + oh_bytes > 150 * 1024
    if not safe:
        x_bufs = max(x_bufs, need_x)
        oh_bufs = max(oh_bufs, need_oh)

    with tile.TileContext(nc) as tc:
        with (
            tc.tile_pool(name="const", bufs=1) as const_pool,
            tc.tile_pool(name="xin", bufs=x_bufs) as x_pool,
            tc.tile_pool(name="oh", bufs=oh_bufs) as oh_pool,
            tc.tile_pool(name="ev", bufs=ev_bufs) as ev_pool,
            tc.tile_pool(name="psum", bufs=ps_bufs, space="PSUM") as psum_pool,
        ):
            iota_i = const_pool.tile([P, IOTA_W], mybir.dt.int32)
            nc.gpsimd.iota(iota_i[:], pattern=[[1, IOTA_W]], base=0, channel_multiplier=0)
            iota_f = const_pool.tile([P, IOTA_W], foh)
            nc.vector.tensor_copy(out=iota_f[:], in_=iota_i[:])

            dummy_x = None
            if not do_in:
                dummy_x = const_pool.tile([P, in_group, H], fin)
                nc.vector.memset(dummy_x[:], 0.0)

            IG, OG = in_group, out_group
            out_eng = nc.sync if out_engine == "sync" else nc.scalar
            ev_eng = nc.vector if ev_engine == "vector" else nc.scalar
            splits = NSPLITS if n_splits is None else n_splits

            def emit(rep):
                for slot in range(SPC):
                    spans, members = plans[slot]
                    wid_eng = nc.sync if wid_engine == "sync" else nc.scalar
                    wid_t = const_pool.tile(
                        [P, KT], foh, name=f"wid_{rep}_{slot}", tag=f"wid{slot}"
                    )
                    wid_eng.dma_start(out=wid_t[:], in_=widf[slot, :, :])
                    if oh_mode == "tt":
                        rcpw_t = const_pool.tile(
                            [P, NW], f32, name=f"rcpw_{rep}_{slot}", tag=f"rcpw{slot}"
                        )
                        wid_eng.dma_start(out=rcpw_t[:], in_=rcpw[slot, :, :])
                    else:
                        rcp_t = const_pool.tile(
                            [P, KT], foh, name=f"rcp_{rep}_{slot}", tag=f"rcp{slot}"
                        )
                        wid_eng.dma_start(out=rcp_t[:], in_=rcp[slot, :, :])

                    xg_tiles = {}
                    oh_tiles = {}

                    def get_x(t):
                        if not do_in:
                            return dummy_x[:, t % IG, :]
                        g, a = divmod(t, IG)
                        if g not in xg_tiles:
                            xt = x_pool.tile(
                                [P, IG, H], fin, name=f"xt_{rep}_{slot}_{g}", tag="xt"
                            )
                            if do_in:
                                if in_layout == "pmajor":
                                    src = x[slot * (KT // IG) + g, :, :].rearrange(
                                        "p (a h) -> p a h", a=IG
                                    )
                                else:
                                    r0 = slot * S + g * IG * P
                                    src = x[r0 : r0 + IG * P, :].rearrange(
                                        "(a p) h -> p a h", p=P
                                    )
                                eng = (
                                    nc.scalar
                                    if (in_alt and g % 2 == 1)
                                    else nc.sync
                                )
                                eng.dma_start(out=xt[:], in_=src)
                            xg_tiles[g] = xt
                        return xg_tiles[g][:, t % IG, :]

                    def get_oh(t):
                        if t not in oh_tiles:
                            jlo, jhi = spans[t]
                            wspan = (jhi - jlo + 1) * P
                            oh = oh_pool.tile(
                                [P, max_span * P],
                                fin,
                                name=f"oh_{rep}_{slot}_{t}",
                                tag="oh",
                            )
                            if oh_mode == "tt":
                                nc.vector.tensor_tensor(
                                    out=oh[:, :wspan],
                                    in0=wid_t[:, t : t + 1].to_broadcast((P, wspan)),
                                    in1=iota_f[:, jlo * P : jlo * P + wspan],
                                    op=mybir.AluOpType.is_equal,
                                )
                            else:
                                nc.vector.tensor_scalar(
                                    out=oh[:, :wspan],
                                    in0=iota_f[:, jlo * P : jlo * P + wspan],
                                    scalar1=wid_t[:, t : t + 1],
                                    scalar2=rcp_t[:, t : t + 1],
                                    op0=mybir.AluOpType.is_equal,
                                    op1=mybir.AluOpType.mult,
                                )
                            oh_tiles[t] = oh
                        return oh_tiles[t]

                    og_tile = [None]

                    for j in range(NW):
                        if do_ev and j % OG == 0:
                            og_tile[0] = ev_pool.tile(
                                [P, OG, H], fout, name=f"out_{rep}_{slot}_{j}", tag="out"
                            )
                        out_sb = og_tile[0][:, j % OG, :] if do_ev else None
                        ks = members[j]
                        if not do_mm:
                            for t in ks:
                                get_x(t)
                                if oh_probe != "skip":
                                    get_oh(t)
                        if not ks:
                            if do_ev:
                                nc.vector.memset(out_sb, 0.0)
                        elif not do_mm:
                            if do_ev and do_out:
                                nc.vector.memset(out_sb, 0.0)
                        else:
                            ps = psum_pool.tile(
                                [P, H], f32, name=f"ps_{rep}_{slot}_{j}", tag="ps"
                            )
                            for ki, t in enumerate(ks):
                                if safe:
                                    xs = x_pool.tile(
                                        [P, 1, H], fin,
                                        name=f"xs_{rep}_{slot}_{j}_{t}", tag="xt",
                                    )
                                    r0 = slot * S + t * P
                                    nc.sync.dma_start(
                                        out=xs[:], in_=x[r0 : r0 + P, :].rearrange(
                                            "(a p) h -> p a h", p=P
                                        )
                                    )
                                    xt = xs[:, 0, :]
                                    oh = oh_pool.tile(
                                        [P, P], fin,
                                        name=f"ohs_{rep}_{slot}_{j}_{t}", tag="oh",
                                    )
                                    if oh_mode == "tt":
                                        nc.vector.tensor_tensor(
                                            out=oh[:, :],
                                            in0=wid_t[:, t : t + 1].to_broadcast(
                                                (P, P)
                                            ),
                                            in1=iota_f[:, j * P : (j + 1) * P],
                                            op=mybir.AluOpType.is_equal,
                                        )
                                    else:
                                        nc.vector.tensor_scalar(
                                            out=oh[:, :],
                                            in0=iota_f[:, j * P : (j + 1) * P],
                                            scalar1=wid_t[:, t : t + 1],
                                            scalar2=rcp_t[:, t : t + 1],
                                            op0=mybir.AluOpType.is_equal,
                                            op1=mybir.AluOpType.mult,
                                        )
                                    off = 0
                                else:
                                    xt = get_x(t)
                                    oh = get_oh(t)
                                    off = (j - spans[t][0]) * P
                                for lo, hi in splits:
                                    nc.tensor.matmul(
                                        out=ps[:, lo:hi],
                                        lhsT=oh[:, off : off + P],
                                        rhs=xt[:, lo:hi],
                                        start=(ki == 0),
                                        stop=(ki == len(ks) - 1),
                                    )
                            if not do_ev:
                                pass
                            elif ev_engine == "vector" or (
                                ev_engine == "alt" and j % 2 == 1
                            ):
                                if oh_mode == "tt":
                                    nc.vector.tensor_tensor(
                                        out=out_sb,
                                        in0=ps[:],
                                        in1=rcpw_t[:, j : j + 1].to_broadcast((P, H)),
                                        op=mybir.AluOpType.mult,
                                    )
                                else:
                                    nc.vector.tensor_copy(out=out_sb, in_=ps[:])
                            else:
                                if oh_mode == "tt":
                                    nc.scalar.activation(
                                        out=out_sb,
                                        in_=ps[:],
                                        func=mybir.ActivationFunctionType.Copy,
                                        scale=rcpw_t[:, j : j + 1],
                                    )
                                else:
                                    nc.scalar.copy(out=out_sb, in_=ps[:])
                        if do_ev and do_out and j % OG == OG - 1:
                            if out_layout == "pmajor":
                                dst = y[slot * (NW // OG) + j // OG, :, :].rearrange(
                                    "p (a h) -> p a h", a=OG
                                )
                            else:
                                r0 = slot * NUM_WORDS + (j - OG + 1) * P
                                dst = y[r0 : r0 + OG * P, :].rearrange(
                                    "(a p) h -> p a h", p=P
                                )
                            oe = (
                                (nc.scalar if (j // OG) % 2 == 0 else nc.sync)
                                if out_engine == "alt"
                                else out_eng
                            )
                            oe.dma_start(out=dst, in_=og_tile[0][:])

            loop_cm = (
                tc.For_i(0, dyn_reps, 1, staggered_reset=stagger)
                if dyn_reps > 1
                else nullcontext()
            )
            with loop_cm:
                for rep in range(reps):
                    emit(rep)

    nc.compile()
    return nc


def _word_recips(word_ids: np.ndarray) -> np.ndarray:
    """[B, P, NW] f32: 1/count for word j*P+p of sample b (1 if empty)."""
    out = np.ones((B, NUM_WORDS), np.float32)
    for b in range(B):
        wid = word_ids[b]
        counts = np.bincount(wid[wid >= 0], minlength=NUM_WORDS)
        nz = counts > 0
        out[b, nz] = 1.0 / counts[nz]
    return np.ascontiguousarray(
        out.reshape(B, NW, P).transpose(0, 2, 1).astype(np.float32)
    )


def _prep_inputs(hidden_states, word_ids, in_dtype="f16", in_layout="rowmajor",
                 in_group=4, oh_dtype="f32", oh_mode="ptr"):
    if in_dtype == "bf16":
        import ml_dtypes

        np_in = ml_dtypes.bfloat16
    else:
        np_in = np.float16 if in_dtype == "f16" else np.float32
    np_oh = np.float16 if oh_dtype == "f16" else np.float32
    hs = np.ascontiguousarray(np.asarray(hidden_states, dtype=np_in))
    wid = np.minimum(np.asarray(word_ids, dtype=np.int32), NUM_WORDS - 1)
    assert hs.shape == (B, S, H) and wid.shape == (B, S)
    r = _recip_counts(wid)
    # [B, S] -> [B, P, KT]: element (p, t) = token t*P + p
    widf = np.ascontiguousarray(
        wid.astype(np_oh).reshape(B, KT, P).transpose(0, 2, 1)
    )
    rt = np.ascontiguousarray(r.astype(np_oh).reshape(B, KT, P).transpose(0, 2, 1))
    rcpw = _word_recips(wid) if oh_mode == "tt" else None
    in_maps = []
    for c in range(N_CORES):
        sl = slice(c * SPC, (c + 1) * SPC)
        if in_layout == "pmajor":
            IG = in_group
            xc = np.ascontiguousarray(
                hs[sl]
                .reshape(SPC, KT // IG, IG, P, H)
                .transpose(0, 1, 3, 2, 4)
                .reshape(SPC * KT // IG, P, IG * H)
            )
        else:
            xc = hs[sl].reshape(SPC * S, H)
        m = {"x": xc, "widf": widf[sl]}
        if oh_mode == "tt":
            m["rcpw"] = rcpw[sl]
        else:
            m["rcp"] = rt[sl]
        in_maps.append(m)
    return in_maps


# config used by kernel() — the graded entry point. `stagger` only affects
# dyn_reps>1 timing builds (cheaper For_i semaphore reset); kernel()'s
# single-shot build has no loop and ignores it.
BUILD_KW = {"oh_mode": "tt", "oh_dtype": "f16", "stagger": True}
PREP_KW = {"oh_mode": "tt", "oh_dtype": "f16"}


def kernel(hidden_states, word_ids):
    import concourse.bass_utils as bass_utils

    wid = np.asarray(word_ids, dtype=np.int32)
    plans = _plan(wid)
    nc = _build(plans, **BUILD_KW)
    in_maps = _prep_inputs(hidden_states, word_ids, **PREP_KW)
    res = bass_utils.run_bass_kernel_spmd(nc, in_maps, core_ids=list(range(N_CORES)))
    out = np.empty((B, NUM_WORDS, H), np.float32)
    OG = BUILD_KW.get("out_group", 2)
    pmajor_out = BUILD_KW.get("out_layout", "rowmajor") == "pmajor"
    for c in range(N_CORES):
        yc = np.asarray(res.results[c]["y"], dtype=np.float32)
        if pmajor_out:
            yc = (
                yc.reshape(SPC, NW // OG, P, OG, H)
                .transpose(0, 1, 3, 2, 4)
                .reshape(SPC, NUM_WORDS, H)
            )
            for slot in range(SPC):
                out[c * SPC + slot] = yc[slot]
        else:
            for slot in range(SPC):
                out[c * SPC + slot] = yc[slot * NUM_WORDS : (slot + 1) * NUM_WORDS]
    return out



# revision 43
# speedup vs baseline: 2.7617x; 1.0056x over previous
"""Segment-mean (word-pooling) kernel for Trainium2, 8 NeuronCores.

Problem: hidden_states [16, 4096, 768] f32, word_ids [16, 4096] i32
(non-decreasing per row, -1 = special token). Output [16, 2048, 768] f32:
mean of each word's subword embeddings; words with no tokens -> 0.

Strategy: pure data parallelism, 2 samples per core. Per sample, the
segment-mean is computed as a banded one-hot matmul on the PE:
  out[w, h] = sum_s onehot[s, w] * (1/count[w]) * x[s, h]
Tokens are processed in 32 k-tiles of 128; since word ids are
non-decreasing, each k-tile only touches a <=128-wide band of words, so
each k-tile contributes 1-2 matmuls into 128-word output windows
accumulated in PSUM. In the default "tt" mode the one-hot is a pure 0/1
mask built on the vector engine with one tensor_tensor is_equal per
k-tile (wid column broadcast along the free dim vs an iota ramp; exact
in f16), and the per-word 1/count is folded into the PSUM->SBUF
eviction as a per-partition activation scale on the scalar engine.
The legacy "ptr" mode fused is_equal*mult with per-partition scalar
pointers on the vector engine instead.

Measured on hw (R256/R2048 amortized): the kernel is DMA-roofline
bound; in+out f16 traffic is 18.9 MB/core and the full kernel time
equals the pure-DMA ablation time, with PE (~52us chain) and
DVE/ACT work fully hidden. Layout variants (pmajor contiguous input,
grouped/pmajor output, queue alternation) all measured neutral-to-worse
than this rowmajor IG=4/OG=2 configuration.

The SPMD program is identical on all 8 cores; the (k-tile, window)
pair structure is the union over samples, so per-core data that doesn't
touch a scheduled pair just contributes a zero one-hot block.
"""

import numpy as np

B, S, H = 16, 4096, 768
NUM_WORDS = S // 2  # 2048
N_CORES = 8
SPC = B // N_CORES  # samples per core = 2
P = 128
KT = S // P  # 32 k-tiles per sample
NW = NUM_WORDS // P  # 16 output windows per sample
NSPLITS = ((0, 512), (512, 768))  # matmul free-dim splits of H


def _plan(word_ids: np.ndarray):
    """Per-slot union plan. For each slot (0/1) and k-tile t: the window
    span [minwin, maxwin] over that slot's 8 samples; per window j the
    sorted member k-tiles. Returns (spans, members) per slot."""
    word_ids = np.minimum(word_ids, NUM_WORDS - 1)
    plans = []
    for slot in range(SPC):
        wid = word_ids[slot::SPC]  # the 8 samples this slot sees
        minwin = np.full(KT, NW, np.int64)
        maxwin = np.full(KT, -1, np.int64)
        for b in range(wid.shape[0]):
            row = wid[b]
            for t in range(KT):
                w = row[t * P : (t + 1) * P]
                w = w[w >= 0]
                if w.size:
                    minwin[t] = min(minwin[t], w.min() // P)
                    maxwin[t] = max(maxwin[t], w.max() // P)
        members = {j: [] for j in range(NW)}
        spans = []
        for t in range(KT):
            if maxwin[t] < 0:  # no valid token anywhere (can't happen)
                spans.append((0, 0))
                continue
            spans.append((int(minwin[t]), int(maxwin[t])))
            for j in range(int(minwin[t]), int(maxwin[t]) + 1):
                members[j].append(t)
        plans.append((spans, members))
    return plans


def _liveness(plans, in_group):
    """Max number of simultaneously-live x DMA groups / onehot tiles over
    the per-window emission order, across slots. A pool needs at least
    this many bufs or slot reuse can deadlock the DMA ring."""
    max_live_g, max_live_oh = 0, 0
    for spans, members in plans:
        first_g, last_g, first_oh, last_oh = {}, {}, {}, {}
        for j in range(NW):
            for t in members[j]:
                g = t // in_group
                first_g.setdefault(g, j)
                last_g[g] = j
                first_oh.setdefault(t, j)
                last_oh[t] = j
        for j in range(NW):
            live_g = sum(1 for g in first_g if first_g[g] <= j <= last_g[g])
            live_oh = sum(1 for t in first_oh if first_oh[t] <= j <= last_oh[t])
            max_live_g = max(max_live_g, live_g)
            max_live_oh = max(max_live_oh, live_oh)
    return max_live_g, max_live_oh


def _recip_counts(word_ids: np.ndarray) -> np.ndarray:
    """Per-token 1/count(word) as f32; 0 for special (-1) tokens."""
    r = np.zeros((B, S), np.float32)
    for b in range(B):
        wid = word_ids[b]
        valid = wid >= 0
        counts = np.bincount(wid[valid], minlength=NUM_WORDS)
        r[b, valid] = (1.0 / counts[wid[valid]]).astype(np.float32)
    return r


def _build(
    plans,
    reps=1,
    dyn_reps=1,
    do_mm=True,
    do_out=True,
    do_in=True,
    do_ev=True,
    x_bufs=8,
    oh_bufs=8,
    ev_bufs=4,
    ps_bufs=3,
    in_group=4,
    out_group=2,
    in_alt=False,
    out_engine="scalar",
    in_dtype="f16",
    out_dtype="f16",
    n_splits=None,
    ev_engine="scalar",
    in_layout="rowmajor",
    out_layout="rowmajor",
    wid_engine="scalar",
    oh_dtype="f32",
    oh_mode="ptr",
    oh_probe=None,
    stagger=False,
    in_engine="sync",
    edge_split=False,
):
    """Build + compile the SPMD Bass program. reps>1 unrolls the whole
    body; dyn_reps>1 wraps it in a hardware For loop — both only used
    for amortized wall-clock timing. do_* flags ablate kernel stages
    for benchmarking (outputs are wrong when any is False)."""
    from contextlib import nullcontext
    import concourse.bacc as bacc
    import concourse.tile as tile
    from concourse import mybir

    nc = bacc.Bacc(
        "TRN2",
        target_bir_lowering=False,
        debug=False,
        enable_asserts=False,
        num_devices=N_CORES,
    )
    f32 = mybir.dt.float32
    _dtmap = {"f16": mybir.dt.float16, "bf16": mybir.dt.bfloat16, "f32": f32}
    fin = _dtmap[in_dtype]
    fout = _dtmap[out_dtype]
    if in_layout == "pmajor":
        x = nc.dram_tensor(
            "x", [SPC * KT // in_group, P, in_group * H], fin, kind="ExternalInput"
        ).ap()
    else:
        x = nc.dram_tensor("x", [SPC * S, H], fin, kind="ExternalInput").ap()
    foh = _dtmap[oh_dtype]
    widf = nc.dram_tensor("widf", [SPC, P, KT], foh, kind="ExternalInput").ap()
    if oh_mode == "tt":
        # per-word reciprocal counts, applied as per-partition scale on evict
        rcpw = nc.dram_tensor("rcpw", [SPC, P, NW], f32, kind="ExternalInput").ap()
    else:
        rcp = nc.dram_tensor("rcp", [SPC, P, KT], foh, kind="ExternalInput").ap()
    if out_layout == "pmajor":
        y = nc.dram_tensor(
            "y", [SPC * NW // out_group, P, out_group * H], fout, kind="ExternalOutput"
        ).ap()
    else:
        y = nc.dram_tensor("y", [SPC * NUM_WORDS, H], fout, kind="ExternalOutput").ap()

    IOTA_W = NUM_WORDS + 2 * P  # ramp long enough for any window pair
    max_span = max(
        (jhi - jlo + 1) for spans, _ in plans for (jlo, jhi) in spans
    )

    # Size pools from plan liveness; degenerate plans (heavily overlapping
    # window k-ranges) fall back to a bounded reload mode, otherwise pool
    # slot reuse can deadlock the DMA ring.
    in_b = 2 if in_dtype == "f16" else 4
    live_g, live_oh = _liveness(plans, in_group)
    need_x, need_oh = live_g + 3, live_oh + 3
    x_bytes = need_x * in_group * H * in_b
    oh_bytes = need_oh * max_span * P * in_b
    safe = x_bytes + oh_bytes > 150 * 1024
    if not safe:
        x_bufs = max(x_bufs, need_x)
        oh_bufs = max(oh_bufs, need_oh)

    with tile.TileContext(nc) as tc:
        with (
            tc.tile_pool(name="const", bufs=1) as const_pool,
            tc.tile_pool(name="xin", bufs=x_bufs) as x_pool,
            tc.tile_pool(name="oh", bufs=oh_bufs) as oh_pool,
            tc.tile_pool(name="ev", bufs=ev_bufs) as ev_pool,
            tc.tile_pool(name="psum", bufs=ps_bufs, space="PSUM") as psum_pool,
        ):
            iota_i = const_pool.tile([P, IOTA_W], mybir.dt.int32)
            nc.gpsimd.iota(iota_i[:], pattern=[[1, IOTA_W]], base=0, channel_multiplier=0)
            iota_f = const_pool.tile([P, IOTA_W], foh)
            nc.vector.tensor_copy(out=iota_f[:], in_=iota_i[:])

            dummy_x = None
            if not do_in:
                dummy_x = const_pool.tile([P, in_group, H], fin)
                nc.vector.memset(dummy_x[:], 0.0)

            IG, OG = in_group, out_group
            if out_engine == "gpsimd":
                out_eng = nc.gpsimd
            else:
                out_eng = nc.sync if out_engine == "sync" else nc.scalar
            ev_eng = nc.vector if ev_engine == "vector" else nc.scalar
            splits = NSPLITS if n_splits is None else n_splits

            def emit(rep):
                for slot in range(SPC):
                    spans, members = plans[slot]
                    wid_eng = nc.sync if wid_engine == "sync" else nc.scalar
                    wid_t = const_pool.tile(
                        [P, KT], foh, name=f"wid_{rep}_{slot}", tag=f"wid{slot}"
                    )
                    wid_eng.dma_start(out=wid_t[:], in_=widf[slot, :, :])
                    if oh_mode == "tt":
                        rcpw_t = const_pool.tile(
                            [P, NW], f32, name=f"rcpw_{rep}_{slot}", tag=f"rcpw{slot}"
                        )
                        wid_eng.dma_start(out=rcpw_t[:], in_=rcpw[slot, :, :])
                    else:
                        rcp_t = const_pool.tile(
                            [P, KT], foh, name=f"rcp_{rep}_{slot}", tag=f"rcp{slot}"
                        )
                        wid_eng.dma_start(out=rcp_t[:], in_=rcp[slot, :, :])

                    xg_tiles = {}
                    oh_tiles = {}

                    def get_x(t):
                        if not do_in:
                            return dummy_x[:, t % IG, :]
                        g, a = divmod(t, IG)
                        if g not in xg_tiles:
                            xt = x_pool.tile(
                                [P, IG, H], fin, name=f"xt_{rep}_{slot}_{g}", tag="xt"
                            )
                            if do_in:
                                if in_layout == "pmajor":
                                    src = x[slot * (KT // IG) + g, :, :].rearrange(
                                        "p (a h) -> p a h", a=IG
                                    )
                                else:
                                    r0 = slot * S + g * IG * P
                                    src = x[r0 : r0 + IG * P, :].rearrange(
                                        "(a p) h -> p a h", p=P
                                    )
                                if in_engine == "gpsimd":
                                    eng = nc.gpsimd
                                elif in_engine == "mix":
                                    eng = nc.gpsimd if g % 2 == 1 else nc.sync
                                else:
                                    eng = (
                                        nc.scalar
                                        if (in_alt and g % 2 == 1)
                                        else nc.sync
                                    )
                                if (
                                    edge_split
                                    and slot == 0
                                    and g == 0
                                    and in_layout == "rowmajor"
                                ):
                                    # split the very first transfer so the
                                    # first k-tiles' matmuls start sooner
                                    h = IG // 2
                                    eng.dma_start(
                                        out=xt[:, :h, :], in_=src[:, :h, :]
                                    )
                                    eng.dma_start(
                                        out=xt[:, h:, :], in_=src[:, h:, :]
                                    )
                                else:
                                    eng.dma_start(out=xt[:], in_=src)
                            xg_tiles[g] = xt
                        return xg_tiles[g][:, t % IG, :]

                    def get_oh(t):
                        if t not in oh_tiles:
                            jlo, jhi = spans[t]
                            wspan = (jhi - jlo + 1) * P
                            oh = oh_pool.tile(
                                [P, max_span * P],
                                fin,
                                name=f"oh_{rep}_{slot}_{t}",
                                tag="oh",
                            )
                            if oh_mode == "tt":
                                nc.vector.tensor_tensor(
                                    out=oh[:, :wspan],
                                    in0=wid_t[:, t : t + 1].to_broadcast((P, wspan)),
                                    in1=iota_f[:, jlo * P : jlo * P + wspan],
                                    op=mybir.AluOpType.is_equal,
                                )
                            else:
                                nc.vector.tensor_scalar(
                                    out=oh[:, :wspan],
                                    in0=iota_f[:, jlo * P : jlo * P + wspan],
                                    scalar1=wid_t[:, t : t + 1],
                                    scalar2=rcp_t[:, t : t + 1],
                                    op0=mybir.AluOpType.is_equal,
                                    op1=mybir.AluOpType.mult,
                                )
                            oh_tiles[t] = oh
                        return oh_tiles[t]

                    og_tile = [None]

                    for j in range(NW):
                        if do_ev and j % OG == 0:
                            og_tile[0] = ev_pool.tile(
                                [P, OG, H], fout, name=f"out_{rep}_{slot}_{j}", tag="out"
                            )
                        out_sb = og_tile[0][:, j % OG, :] if do_ev else None
                        ks = members[j]
                        if not do_mm:
                            for t in ks:
                                get_x(t)
                                if oh_probe != "skip":
                                    get_oh(t)
                        if not ks:
                            if do_ev:
                                nc.vector.memset(out_sb, 0.0)
                        elif not do_mm:
                            if do_ev and do_out:
                                nc.vector.memset(out_sb, 0.0)
                        else:
                            ps = psum_pool.tile(
                                [P, H], f32, name=f"ps_{rep}_{slot}_{j}", tag="ps"
                            )
                            for ki, t in enumerate(ks):
                                if safe:
                                    xs = x_pool.tile(
                                        [P, 1, H], fin,
                                        name=f"xs_{rep}_{slot}_{j}_{t}", tag="xt",
                                    )
                                    r0 = slot * S + t * P
                                    nc.sync.dma_start(
                                        out=xs[:], in_=x[r0 : r0 + P, :].rearrange(
                                            "(a p) h -> p a h", p=P
                                        )
                                    )
                                    xt = xs[:, 0, :]
                                    oh = oh_pool.tile(
                                        [P, P], fin,
                                        name=f"ohs_{rep}_{slot}_{j}_{t}", tag="oh",
                                    )
                                    if oh_mode == "tt":
                                        nc.vector.tensor_tensor(
                                            out=oh[:, :],
                                            in0=wid_t[:, t : t + 1].to_broadcast(
                                                (P, P)
                                            ),
                                            in1=iota_f[:, j * P : (j + 1) * P],
                                            op=mybir.AluOpType.is_equal,
                                        )
                                    else:
                                        nc.vector.tensor_scalar(
                                            out=oh[:, :],
                                            in0=iota_f[:, j * P : (j + 1) * P],
                                            scalar1=wid_t[:, t : t + 1],
                                            scalar2=rcp_t[:, t : t + 1],
                                            op0=mybir.AluOpType.is_equal,
                                            op1=mybir.AluOpType.mult,
                                        )
                                    off = 0
                                else:
                                    xt = get_x(t)
                                    oh = get_oh(t)
                                    off = (j - spans[t][0]) * P
                                for lo, hi in splits:
                                    nc.tensor.matmul(
                                        out=ps[:, lo:hi],
                                        lhsT=oh[:, off : off + P],
                                        rhs=xt[:, lo:hi],
                                        start=(ki == 0),
                                        stop=(ki == len(ks) - 1),
                                    )
                            if not do_ev:
                                pass
                            elif ev_engine == "vector" or (
                                ev_engine == "alt" and j % 2 == 1
                            ):
                                if oh_mode == "tt":
                                    nc.vector.tensor_tensor(
                                        out=out_sb,
                                        in0=ps[:],
                                        in1=rcpw_t[:, j : j + 1].to_broadcast((P, H)),
                                        op=mybir.AluOpType.mult,
                                    )
                                else:
                                    nc.vector.tensor_copy(out=out_sb, in_=ps[:])
                            else:
                                if oh_mode == "tt":
                                    nc.scalar.activation(
                                        out=out_sb,
                                        in_=ps[:],
                                        func=mybir.ActivationFunctionType.Copy,
                                        scale=rcpw_t[:, j : j + 1],
                                    )
                                else:
                                    nc.scalar.copy(out=out_sb, in_=ps[:])
                        tail_split = (
                            edge_split
                            and slot == SPC - 1
                            and j >= NW - OG
                            and out_layout == "rowmajor"
                        )
                        if do_ev and do_out and tail_split:
                            # flush the final group per window: the last
                            # out transfer starts right after its eviction
                            # instead of waiting for the whole group
                            r0 = slot * NUM_WORDS + j * P
                            dst = y[r0 : r0 + P, :].rearrange(
                                "(a p) h -> p a h", p=P
                            )
                            out_eng.dma_start(
                                out=dst, in_=og_tile[0][:, j % OG : j % OG + 1, :]
                            )
                        elif do_ev and do_out and j % OG == OG - 1:
                            if out_layout == "pmajor":
                                dst = y[slot * (NW // OG) + j // OG, :, :].rearrange(
                                    "p (a h) -> p a h", a=OG
                                )
                            else:
                                r0 = slot * NUM_WORDS + (j - OG + 1) * P
                                dst = y[r0 : r0 + OG * P, :].rearrange(
                                    "(a p) h -> p a h", p=P
                                )
                            oe = (
                                (nc.scalar if (j // OG) % 2 == 0 else nc.sync)
                                if out_engine == "alt"
                                else out_eng
                            )
                            oe.dma_start(out=dst, in_=og_tile[0][:])

            loop_cm = (
                tc.For_i(0, dyn_reps, 1, staggered_reset=stagger)
                if dyn_reps > 1
                else nullcontext()
            )
            with loop_cm:
                for rep in range(reps):
                    emit(rep)

    nc.compile()
    return nc


def _word_recips(word_ids: np.ndarray) -> np.ndarray:
    """[B, P, NW] f32: 1/count for word j*P+p of sample b (1 if empty)."""
    out = np.ones((B, NUM_WORDS), np.float32)
    for b in range(B):
        wid = word_ids[b]
        counts = np.bincount(wid[wid >= 0], minlength=NUM_WORDS)
        nz = counts > 0
        out[b, nz] = 1.0 / counts[nz]
    return np.ascontiguousarray(
        out.reshape(B, NW, P).transpose(0, 2, 1).astype(np.float32)
    )


def _prep_inputs(hidden_states, word_ids, in_dtype="f16", in_layout="rowmajor",
                 in_group=4, oh_dtype="f32", oh_mode="ptr"):
    if in_dtype == "bf16":
        import ml_dtypes

        np_in = ml_dtypes.bfloat16
    else:
        np_in = np.float16 if in_dtype == "f16" else np.float32
    np_oh = np.float16 if oh_dtype == "f16" else np.float32
    hs = np.ascontiguousarray(np.asarray(hidden_states, dtype=np_in))
    wid = np.minimum(np.asarray(word_ids, dtype=np.int32), NUM_WORDS - 1)
    assert hs.shape == (B, S, H) and wid.shape == (B, S)
    r = _recip_counts(wid)
    # [B, S] -> [B, P, KT]: element (p, t) = token t*P + p
    widf = np.ascontiguousarray(
        wid.astype(np_oh).reshape(B, KT, P).transpose(0, 2, 1)
    )
    rt = np.ascontiguousarray(r.astype(np_oh).reshape(B, KT, P).transpose(0, 2, 1))
    rcpw = _word_recips(wid) if oh_mode == "tt" else None
    in_maps = []
    for c in range(N_CORES):
        sl = slice(c * SPC, (c + 1) * SPC)
        if in_layout == "pmajor":
            IG = in_group
            xc = np.ascontiguousarray(
                hs[sl]
                .reshape(SPC, KT // IG, IG, P, H)
                .transpose(0, 1, 3, 2, 4)
                .reshape(SPC * KT // IG, P, IG * H)
            )
        else:
            xc = hs[sl].reshape(SPC * S, H)
        m = {"x": xc, "widf": widf[sl]}
        if oh_mode == "tt":
            m["rcpw"] = rcpw[sl]
        else:
            m["rcp"] = rt[sl]
        in_maps.append(m)
    return in_maps


# config used by kernel() — the graded entry point. `stagger` only affects
# dyn_reps>1 timing builds (cheaper For_i semaphore reset); kernel()'s
# single-shot build has no loop and ignores it.
BUILD_KW = {
    "oh_mode": "tt",
    "oh_dtype": "f16",
    "stagger": True,
    "edge_split": True,
}
PREP_KW = {"oh_mode": "tt", "oh_dtype": "f16"}


def kernel(hidden_states, word_ids):
    import concourse.bass_utils as bass_utils

    wid = np.asarray(word_ids, dtype=np.int32)
    plans = _plan(wid)
    nc = _build(plans, **BUILD_KW)
    in_maps = _prep_inputs(hidden_states, word_ids, **PREP_KW)
    res = bass_utils.run_bass_kernel_spmd(nc, in_maps, core_ids=list(range(N_CORES)))
    out = np.empty((B, NUM_WORDS, H), np.float32)
    OG = BUILD_KW.get("out_group", 2)
    pmajor_out = BUILD_KW.get("out_layout", "rowmajor") == "pmajor"
    for c in range(N_CORES):
        yc = np.asarray(res.results[c]["y"], dtype=np.float32)
        if pmajor_out:
            yc = (
                yc.reshape(SPC, NW // OG, P, OG, H)
                .transpose(0, 1, 3, 2, 4)
                .reshape(SPC, NUM_WORDS, H)
            )
            for slot in range(SPC):
                out[c * SPC + slot] = yc[slot]
        else:
            for slot in range(SPC):
                out[c * SPC + slot] = yc[slot * NUM_WORDS : (slot + 1) * NUM_WORDS]
    return out

